# revision 2
# baseline (speedup 1.0000x reference)
import sys

sys.path.insert(0, "/opt/trn_rl_repo")

import numpy as np

X_DIM, Z_DIM, H_DIM, G_DIM, I_DIM = 1, 16, 16, 8, 16
T_LEN, BATCH, N_CORES = 512, 2048, 8
BP = BATCH // N_CORES

MM_DTYPE = "f32r"

_CACHE = {}


def _f(x):
    return np.ascontiguousarray(x, dtype=np.float32)


def _prep_weights(inp):
    W = {}
    Wih = np.asarray(inp["lstm_Wih"]).astype(np.float64)
    Whh = np.asarray(inp["lstm_Whh"]).astype(np.float64)
    bsum = np.asarray(inp["lstm_bih"]).astype(np.float64) + np.asarray(
        inp["lstm_bhh"]
    ).astype(np.float64)
    gsrc = {"i": slice(0, 16), "f": slice(16, 32), "g": slice(32, 48), "o": slice(48, 64)}
    gdst = {"i": 0, "f": 32, "o": 64, "g": 96}
    lstm_h = np.zeros((17, 112), np.float64)
    lstm_x = np.zeros((33, 112), np.float64)
    for k in ("i", "f", "o", "g"):
        d = gdst[k]
        lstm_h[0:16, d : d + 16] = Whh[gsrc[k], :].T
        lstm_h[16, d : d + 16] = bsum[gsrc[k]]
        lstm_x[32, d : d + 16] = Wih[gsrc[k], 0]
    W["lstm_h"] = _f(lstm_h)
    W["lstm_x"] = _f(lstm_x)

    cW1 = np.asarray(inp["comb_W1"]).astype(np.float64)
    comb1h = np.zeros((49, 16), np.float64)
    comb1h[32:48, :] = cW1[:, 0:16].T
    comb1h[48, :] = np.asarray(inp["comb_b1"]).astype(np.float64)
    W["comb1h"] = _f(comb1h)
    comb1z = np.zeros((48, 16), np.float64)
    comb1z[0:16, :] = cW1[:, 16:32].T
    comb1z[32:48, :] = cW1[:, 16:32].T
    W["comb1z"] = _f(comb1z)

    eW1 = np.asarray(inp["enc_W1"]).astype(np.float64)
    cW2 = np.asarray(inp["comb_W2"]).astype(np.float64)
    Wp = eW1 @ cW2
    bp = eW1 @ np.asarray(inp["comb_b2"]).astype(np.float64) + np.asarray(
        inp["enc_b1"]
    ).astype(np.float64)
    wp_lhsT = np.zeros((33, 16), np.float64)
    wp_lhsT[0:16, :] = Wp.T
    wp_lhsT[32, :] = bp
    W["wp_lhsT"] = _f(wp_lhsT)

    eW2 = np.asarray(inp["enc_W2"]).astype(np.float64)
    eb2 = np.asarray(inp["enc_b2"]).astype(np.float64)
    enc2_lhsT = np.zeros((33, 48), np.float64)
    enc2_lhsT[0:16, 0:16] = eW2[0:16, :].T
    enc2_lhsT[32, 0:16] = eb2[0:16]
    enc2_lhsT[0:16, 32:48] = eW2[16:32, :].T
    enc2_lhsT[32, 32:48] = eb2[16:32]
    W["enc2_lhsT"] = _f(enc2_lhsT)

    dW1 = np.asarray(inp["dec_W1"]).astype(np.float64)
    dec1bd = np.zeros((128, 128), np.float64)
    for g in range(8):
        dec1bd[16 * g : 16 * g + 16, 16 * g : 16 * g + 16] = dW1.T
    W["dec1bd"] = _f(dec1bd)
    W["dec1b"] = _f(np.tile(np.asarray(inp["dec_b1"]), 8).reshape(128, 1))
    dW2 = np.asarray(inp["dec_W2"]).astype(np.float64)
    dec2bd = np.zeros((128, 16), np.float64)
    for g in range(8):
        dec2bd[16 * g : 16 * g + 16, 2 * g : 2 * g + 2] = dW2.T
    W["dec2bd"] = _f(dec2bd)
    W["dec2b"] = _f(np.tile(np.asarray(inp["dec_b2"]), 8).reshape(16, 1))

    tW1 = np.asarray(inp["tr_W1"]).astype(np.float64)
    tr1bd = np.zeros((128, 128), np.float64)
    for g in range(1, 8):
        tr1bd[16 * (g - 1) : 16 * g, 16 * g : 16 * g + 16] = tW1.T
    W["tr1bd"] = _f(tr1bd)
    W["tr1g0"] = _f(tW1.T)
    W["tr1b"] = _f(np.tile(np.asarray(inp["tr_b1"]), 8).reshape(128, 1))
    tW2 = np.asarray(inp["tr_W2"]).astype(np.float64)
    tr2bd = np.zeros((128, 128), np.float64)
    for h in range(4):
        tr2bd[16 * h : 16 * h + 16, 32 * h : 32 * h + 32] = tW2.T
        tr2bd[64 + 16 * h : 64 + 16 * h + 16, 32 * h : 32 * h + 32] = tW2.T
    W["tr2bd"] = _f(tr2bd)
    W["tr2b"] = _f(np.tile(np.asarray(inp["tr_b2"]), 4).reshape(128, 1))
    return W


WEIGHT_SHAPES = {
    "lstm_h": (17, 112),
    "lstm_x": (33, 112),
    "comb1h": (49, 16),
    "comb1z": (48, 16),
    "wp_lhsT": (33, 16),
    "enc2_lhsT": (33, 48),
    "dec1bd": (128, 128),
    "dec1b": (128, 1),
    "dec2bd": (128, 16),
    "dec2b": (16, 1),
    "tr1bd": (128, 128),
    "tr1g0": (16, 16),
    "tr1b": (128, 1),
    "tr2bd": (128, 128),
    "tr2b": (128, 1),
}


def build_nc(T=T_LEN, mm_dtype=MM_DTYPE):
    import concourse.bacc as bacc
    import concourse.tile as tile
    from concourse import mybir

    f32 = mybir.dt.float32
    f32r = mybir.dt.float32r

    def _bc(ap):
        return ap.bitcast(f32r) if mm_dtype == "f32r" else ap

    AF = mybir.ActivationFunctionType
    NM = T // 16
    NB8 = T // 8

    nc = bacc.Bacc(None)

    x_in = nc.dram_tensor("x_r", [T, BP], f32, kind="ExternalInput")
    eps_in = nc.dram_tensor("eps_fm", [Z_DIM, T, BP], f32, kind="ExternalInput")
    wdram = {}
    for name, shape in WEIGHT_SHAPES.items():
        wdram[name] = nc.dram_tensor(name, list(shape), f32, kind="ExternalInput")

    out_e = nc.dram_tensor("out_e", [32, T, BP], f32, kind="ExternalOutput")
    out_d = nc.dram_tensor("out_d", [NM, 16, 512], f32, kind="ExternalOutput")
    out_t = nc.dram_tensor("out_t", [NM, 2, 128, 512], f32, kind="ExternalOutput")

    HB = BP // 2
    chs = [slice(0, HB), slice(HB, BP)]

    with tile.TileContext(nc) as tc:
        with tc.tile_pool(name="persist", bufs=1) as pp, tc.tile_pool(
            name="dstage", bufs=1, space="DRAM"
        ) as dp:
            hseq_d = dp.tile([16, T, BP], f32, name="hseq_d", tag="hseq_d")
            zpk_d = dp.tile([16, T, BP], f32, name="zpk_d", tag="zpk_d")

            wt = {}
            for name, shape in WEIGHT_SHAPES.items():
                wt[name] = pp.tile(list(shape), f32, name=f"w_{name}", tag=f"w_{name}")
                nc.sync.dma_start(out=wt[name][:, :], in_=wdram[name][:, :])

            cst = pp.tile([16, BP], f32, name="c_state", tag="c_state")
            nc.vector.memset(cst[:, :], 0.0)
            zzero = pp.tile([16, BP], f32, name="zzero", tag="zzero")
            nc.vector.memset(zzero[:, :], 0.0)

            hstgA = []
            for r in range(2):
                t_ = pp.tile([17, 8 * BP], f32, name=f"hstgA{r}", tag=f"hstgA{r}")
                nc.vector.memset(t_[0:16, :], 0.0)
                nc.vector.memset(t_[16:17, :], 1.0)
                hstgA.append(t_)
            xstgA = pp.tile([33, 3 * 16 * BP], f32, name="xstgA", tag="xstgA")

            hstgB = []
            for r in range(3):
                t_ = pp.tile([49, 8 * BP], f32, name=f"hstgB{r}", tag=f"hstgB{r}")
                nc.vector.memset(t_[48:49, :], 1.0)
                hstgB.append(t_)
            epsst = [
                pp.tile([16, 8 * BP], f32, name=f"epsst{r}", tag=f"epsst{r}")
                for r in range(3)
            ]
            estg = [
                pp.tile([48, 8 * BP], f32, name=f"estg{r}", tag=f"estg{r}")
                for r in range(2)
            ]
            zstg = [
                pp.tile([16, 8 * BP], f32, name=f"zstg{r}", tag=f"zstg{r}")
                for r in range(2)
            ]
            RB = 3
            zrhs = []
            for r in range(RB):
                t_ = pp.tile([48, BP], f32, name=f"zrhs{r}", tag=f"zrhs{r}")
                nc.vector.memset(t_[0:48, :], 0.0)
                zrhs.append(t_)

            with (
                tc.tile_pool(name="apsum", bufs=2, space="PSUM") as psA,
                tc.tile_pool(name="asb", bufs=3) as sA,
            ):
                def load_x(blk):
                    xs = slice(16 * BP * (blk % 3), 16 * BP * (blk % 3) + 16 * BP)
                    nc.sync.dma_start(
                        out=xstgA[32:33, xs], in_=x_in[16 * blk : 16 * blk + 16, :]
                    )

                load_x(31)
                load_x(30)

                def a_stage1(t, ci, cs):
                    bb1 = (t + 1) // 8
                    col1 = ((t + 1) % 8) * BP
                    hsrc = hstgA[bb1 % 2] if bb1 < NB8 else hstgA[0]
                    xb = t // 16
                    xcol = 16 * BP * (xb % 3) + (t % 16) * BP
                    pg = psA.tile([112, HB], f32, name=f"pg{ci}", tag=f"pg{ci}", bufs=2)
                    nc.tensor.matmul(
                        pg[:, :],
                        _bc(wt["lstm_h"][:, :]),
                        _bc(hsrc[0:17, col1 + cs.start : col1 + cs.stop]),
                        start=True,
                        stop=False,
                        skip_group_check=True,
                    )
                    nc.tensor.matmul(
                        pg[:, :],
                        _bc(wt["lstm_x"][32:33, :]),
                        _bc(xstgA[32:33, xcol + cs.start : xcol + cs.stop]),
                        start=False,
                        stop=True,
                        tile_position=(32, 0),
                        skip_group_check=True,
                    )
                    sact = sA.tile([80, HB], f32, name=f"sact{ci}", tag=f"sact{ci}")
                    nc.scalar.activation(sact[:, :], pg[0:80, :], AF.Sigmoid)
                    tg = sA.tile([16, HB], f32, name=f"tg{ci}", tag=f"tg{ci}")
                    nc.scalar.activation(tg[:, :], pg[96:112, :], AF.Tanh)
                    return sact, tg

                def a_stage2(t, ci, cs, sact, tg):
                    u = sA.tile([16, HB], f32, name=f"u{ci}", tag=f"u{ci}")
                    nc.vector.tensor_mul(u[:, :], sact[32:48, :], cst[:, cs])
                    v = sA.tile([16, HB], f32, name=f"v{ci}", tag=f"v{ci}")
                    nc.vector.tensor_mul(v[:, :], sact[0:16, :], tg[:, :])
                    nc.vector.tensor_add(cst[:, cs], u[:, :], v[:, :])

                def a_stage3(t, ci, cs, sact):
                    th = sA.tile([16, HB], f32, name=f"th{ci}", tag=f"th{ci}")
                    nc.scalar.activation(th[:, :], cst[:, cs], AF.Tanh)
                    bb = t // 8
                    col = (t % 8) * BP
                    nc.vector.tensor_mul(
                        hstgA[bb % 2][0:16, col + cs.start : col + cs.stop],
                        sact[64:80, :],
                        th[:, :],
                    )

                for j in range(T):
                    t = T - 1 - j
                    s0, g0 = a_stage1(t, 0, chs[0])
                    s1, g1 = a_stage1(t, 1, chs[1])
                    a_stage2(t, 0, chs[0], s0, g0)
                    a_stage2(t, 1, chs[1], s1, g1)
                    a_stage3(t, 0, chs[0], s0)
                    a_stage3(t, 1, chs[1], s1)
                    if t % 16 == 0 and t >= 32:
                        load_x(t // 16 - 2)
                    if t % 8 == 0:
                        bb = t // 8
                        nc.sync.dma_start(
                            out=hseq_d[:, t : t + 8, :], in_=hstgA[bb % 2][0:16, :]
                        )

            with (
                tc.tile_pool(name="bpsum", bufs=2, space="PSUM") as psB,
                tc.tile_pool(name="bsb", bufs=3) as sB,
                tc.tile_pool(name="cpsum", bufs=1, space="PSUM") as psC,
                tc.tile_pool(name="csb", bufs=2) as sC,
            ):
                def load_hb(blk):
                    nc.sync.dma_start(
                        out=hstgB[blk % 3][32:48, :],
                        in_=hseq_d[:, 8 * blk : 8 * blk + 8, :],
                    )

                def load_eps(blk):
                    nc.sync.dma_start(
                        out=epsst[blk % 3][0:16, :],
                        in_=eps_in[:, 8 * blk : 8 * blk + 8, :],
                    )

                load_hb(0)
                load_eps(0)
                load_hb(1)
                load_eps(1)

                def b_stage1(t, ci, cs):
                    bb = t // 8
                    col = (t % 8) * BP
                    p1 = psB.tile([16, HB], f32, name=f"p1{ci}", tag=f"p1{ci}", bufs=2)
                    nc.tensor.matmul(
                        p1[:, :],
                        _bc(wt["comb1h"][32:49, :]),
                        _bc(hstgB[bb % 3][32:49, col + cs.start : col + cs.stop]),
                        start=True,
                        stop=(t == 0),
                        tile_position=(32, 0),
                        skip_group_check=True,
                    )
                    if t > 0:
                        nc.tensor.matmul(
                            p1[:, :],
                            _bc(wt["comb1z"][0:48, :]),
                            _bc(zrhs[t % RB][0:48, cs]),
                            start=False,
                            stop=True,
                            skip_group_check=True,
                        )
                    bt1 = sB.tile([33, HB], f32, name=f"bt1{ci}", tag=f"bt1{ci}")
                    if t == 0:
                        nc.vector.memset(bt1[16:32, :], 0.0)
                        nc.vector.memset(bt1[32:33, :], 1.0)
                    nc.scalar.activation(bt1[0:16, :], p1[:, :], AF.Tanh)
                    return bt1

                def b_stage2(t, ci, cs, bt1):
                    p2 = psB.tile([16, HB], f32, name=f"p2{ci}", tag=f"p2{ci}", bufs=2)
                    nc.tensor.matmul(
                        p2[:, :],
                        _bc(wt["wp_lhsT"][:, :]),
                        _bc(bt1[0:33, :]),
                        start=True,
                        stop=True,
                    )
                    be1 = sB.tile([33, HB], f32, name=f"be1{ci}", tag=f"be1{ci}")
                    if t == 0:
                        nc.vector.memset(be1[16:32, :], 0.0)
                        nc.vector.memset(be1[32:33, :], 1.0)
                    nc.scalar.activation(be1[0:16, :], p2[:, :], AF.Tanh)
                    return be1

                def b_stage3(t, ci, cs, be1):
                    rn = (t + 1) % RB
                    bb = t // 8
                    col = (t % 8) * BP
                    p3 = psB.tile([48, HB], f32, name=f"p3{ci}", tag=f"p3{ci}", bufs=2)
                    nc.tensor.matmul(
                        p3[:, :],
                        _bc(wt["enc2_lhsT"][:, :]),
                        _bc(be1[0:33, :]),
                        start=True,
                        stop=True,
                    )
                    E = sB.tile([16, HB], f32, name=f"E{ci}", tag=f"E{ci}")
                    nc.scalar.activation(E[:, :], p3[32:48, :], AF.Exp, scale=0.5)
                    nc.vector.tensor_mul(
                        zrhs[rn][32:48, cs],
                        epsst[bb % 3][0:16, col + cs.start : col + cs.stop],
                        E[:, :],
                    )
                    nc.vector.tensor_copy(zrhs[rn][0:16, cs], p3[0:16, :])
                    nc.vector.tensor_copy(
                        estg[bb % 2][0:48, col + cs.start : col + cs.stop], p3[:, :]
                    )
                    nc.vector.tensor_add(
                        zstg[bb % 2][0:16, col + cs.start : col + cs.stop],
                        p3[0:16, :],
                        zrhs[rn][32:48, cs],
                    )

                def c_subblock(c):
                    mj, jj = c // 2, c % 2
                    ocol = slice(256 * jj, 256 * jj + 256)
                    zc = sC.tile([128, 256], f32, name="zc", tag="zc", bufs=2)
                    nc.sync.dma_start(out=zc[:, :], in_=zpk_d[:, 8 * c : 8 * c + 8, :])
                    pd1 = psC.tile([128, 256], f32, name="pd1", tag="pd1")
                    nc.tensor.matmul(
                        pd1[:, :], _bc(wt["dec1bd"][:, :]), _bc(zc[:, :]),
                        start=True, stop=True,
                    )
                    d1 = sC.tile([128, 256], f32, name="d1", tag="d1")
                    nc.scalar.activation(
                        d1[:, :], pd1[:, :], AF.Tanh, bias=wt["dec1b"][:, :]
                    )
                    pd2 = psC.tile([16, 256], f32, name="pd2", tag="pd2")
                    nc.tensor.matmul(
                        pd2[:, :], _bc(wt["dec2bd"][:, :]), _bc(d1[:, :]),
                        start=True, stop=True,
                    )
                    dstg = sC.tile([16, 256], f32, name="dstg", tag="dstg")
                    nc.vector.tensor_scalar_add(dstg[:, :], pd2[:, :], wt["dec2b"][:, :])
                    nc.sync.dma_start(out=out_d[mj, :, ocol], in_=dstg[:, :])
                    ptr = psC.tile([128, 256], f32, name="ptr", tag="ptr")
                    nc.tensor.matmul(
                        ptr[:, :], _bc(wt["tr1bd"][:, :]), _bc(zc[:, :]),
                        start=True, stop=False, skip_group_check=True,
                    )
                    if c == 0:
                        prhs = zzero[:, :]
                    else:
                        zf = sC.tile([16, 256], f32, name="zf", tag="zf", bufs=2)
                        nc.sync.dma_start(out=zf[:, :], in_=zpk_d[:, 8 * c - 1, :])
                        prhs = zf[:, :]
                    nc.tensor.matmul(
                        ptr[0:16, :], _bc(wt["tr1g0"][:, :]), _bc(prhs),
                        start=False, stop=True, skip_group_check=True,
                    )
                    d1t = sC.tile([128, 256], f32, name="d1t", tag="d1t")
                    nc.scalar.activation(
                        d1t[:, :], ptr[:, :], AF.Tanh, bias=wt["tr1b"][:, :]
                    )
                    for half in range(2):
                        pt2 = psC.tile(
                            [128, 256], f32, name=f"pt2{half}", tag=f"pt2{half}"
                        )
                        nc.tensor.matmul(
                            pt2[:, :],
                            _bc(wt["tr2bd"][64 * half : 64 * half + 64, :]),
                            _bc(d1t[64 * half : 64 * half + 64, :]),
                            start=True,
                            stop=True,
                        )
                        tstg = sC.tile(
                            [128, 256], f32, name=f"tstg{half}", tag=f"tstg{half}"
                        )
                        nc.vector.tensor_scalar_add(
                            tstg[:, :], pt2[:, :], wt["tr2b"][:, :]
                        )
                        nc.sync.dma_start(out=out_t[mj, half, :, ocol], in_=tstg[:, :])

                for t in range(T):
                    b1c0 = b_stage1(t, 0, chs[0])
                    b1c1 = b_stage1(t, 1, chs[1])
                    b2c0 = b_stage2(t, 0, chs[0], b1c0)
                    b2c1 = b_stage2(t, 1, chs[1], b1c1)
                    b_stage3(t, 0, chs[0], b2c0)
                    b_stage3(t, 1, chs[1], b2c1)
                    if t % 8 == 7:
                        bb = t // 8
                        nc.sync.dma_start(
                            out=out_e[0:16, t - 7 : t + 1, :], in_=estg[bb % 2][0:16, :]
                        )
                        nc.sync.dma_start(
                            out=out_e[16:32, t - 7 : t + 1, :],
                            in_=estg[bb % 2][32:48, :],
                        )
                        nc.sync.dma_start(
                            out=zpk_d[:, t - 7 : t + 1, :], in_=zstg[bb % 2][0:16, :]
                        )
                        if bb + 2 < NB8:
                            load_hb(bb + 2)
                            load_eps(bb + 2)
                        c_subblock(bb)

    nc.finalize()
    return nc


def decode_outputs(res, T=T_LEN):
    NM = T // 16
    oe = res["out_e"]
    mu_z = oe[0:16].transpose(1, 2, 0)
    lv_z = oe[16:32].transpose(1, 2, 0)
    od = res["out_d"].reshape(NM, 8, 2, 2, BP)
    od = od.transpose(0, 3, 1, 2, 4).reshape(T, 2, BP)
    mu_x = od[:, 0, :][:, :, None]
    lv_x = od[:, 1, :][:, :, None]
    ot = res["out_t"].reshape(NM, 2, 4, 32, 2, BP)
    ot = ot.transpose(0, 4, 1, 2, 3, 5).reshape(T, 32, BP)
    mu_t = ot[:, 0:16, :].transpose(0, 2, 1)
    lv_t = ot[:, 16:32, :].transpose(0, 2, 1)
    return np.concatenate([mu_x, lv_x, mu_z, lv_z, mu_t, lv_t], axis=-1)


def make_in_maps(inputs, T=T_LEN, mm_dtype=None):
    W = _prep_weights(inputs)
    x = np.asarray(inputs["x"])
    eps = np.asarray(inputs["eps_z"])
    in_maps = []
    for c in range(N_CORES):
        sl = slice(BP * c, BP * c + BP)
        m = dict(W)
        m["x_r"] = _f(x[:T, sl, 0])
        m["eps_fm"] = _f(eps[:T, sl, :].transpose(2, 0, 1))
        in_maps.append(m)
    return in_maps


def kernel(**inputs):
    from concourse.bass_utils import run_bass_kernel_spmd

    key = (T_LEN, MM_DTYPE)
    if key not in _CACHE:
        _CACHE[key] = build_nc(T_LEN, MM_DTYPE)
    nc = _CACHE[key]
    in_maps = make_in_maps(inputs, T_LEN, MM_DTYPE)
    res = run_bass_kernel_spmd(nc, in_maps, list(range(N_CORES)))
    outs = [decode_outputs(r, T_LEN) for r in res.results]
    return np.concatenate(outs, axis=1)
